# revision 1
# baseline (speedup 1.0000x reference)
"""Causal single-head attention (B=4, S=4096, D=1024, d_key=64) on 8 trn2 cores.

Sharding: 8 cores = 4 batches x 2 query-interleave halves. Core (b, h) handles
batch b and query chunks of 256 rows at global chunk indices {2g+h : g=0..7}
(interleaved for causal load balance). Keys/values for the batch are replicated
on both cores of the pair.

Device kernel (identical SPMD program; per-core differences are input data):
  1. Project kT [64, S] and qT [64, 2048] (weights as lhsT) and v-natural
     [128, 65] blocks (data chunk as lhsT, weights as rhs -- emits the PV
     layout directly, no transposes; a DMA'd ones column provides softmax
     denominators) from host-pre-transposed bf16 KT/VT/QT [1024, S] and
     W*T [1024, 64] (bf16 matmuls, fp32 accumulate). kT/qT are stored fp32r
     so attention matmuls run at full precision-speed (1 cycle/row, N>=256).
  2. KEY-MAJOR attention: for each key group t (512 keys = one "quad" of 4
     key blocks), right after k/v group t is projected, every query chunk
     g >= t computes its transposed scores sT[j, i] = k_j . q_i over that
     group (one [128, 1024] PSUM tile / one ACT exp with scale=1/8 per quad),
     applies the causal boundary mask (host-built multiplicative [128, 1024] tile) when t == g,
     then PV-accumulates the quad into PSUM [65, 256] and DVE-adds it into a
     per-chunk SBUF accumulator osb[:, g, :] whose row 64 is the softmax
     denominator. Only quad (7,7) depends on the final input DMA, so the
     post-DMA tail is tiny.
  3. Per chunk, right after its boundary group: DMA the raw accumulator to
     DRAM via the POOL DGE; the host divides by the denominator row and
     transposes (a few MB of numpy).
"""

import numpy as np

import concourse.mybir as mybir
import concourse.tile as tile
from concourse import bacc
from concourse.bass_utils import run_bass_kernel_spmd

B, S, D, DK = 4, 4096, 1024, 64
NCORES = 8
CH = 256  # query rows per chunk
NCH = 8  # chunks per core
QROWS = CH * NCH  # 2048 query rows per core
JB = 128  # key block
DC = D // 128  # 8 contraction chunks
F32 = mybir.dt.float32
F32R = mybir.dt.float32r
BF16 = mybir.dt.bfloat16

_prog_cache = {}
_last_in_maps = None


def _build(variant):
    causal = variant == "causal"
    # number of key quads (4 key blocks of 128 = 512 keys) per chunk
    nq = [g + 1 for g in range(NCH)] if causal else [S // 512] * NCH

    nc = bacc.Bacc("TRN2", target_bir_lowering=False, debug=False,
                   num_devices=NCORES)

    qt_d = nc.declare_dram_parameter("qt", [D, QROWS], BF16, isOutput=False)
    kt_d = nc.declare_dram_parameter("kt", [D, S], BF16, isOutput=False)
    vt_d = nc.declare_dram_parameter("vt", [D, S], BF16, isOutput=False)
    wq_d = nc.declare_dram_parameter("wq", [D, DK], BF16, isOutput=False)
    wk_d = nc.declare_dram_parameter("wk", [D, DK], BF16, isOutput=False)
    wv_d = nc.declare_dram_parameter("wv", [D, DK], BF16, isOutput=False)
    if causal:
        mask_d = nc.declare_dram_parameter("maskq", [JB, 4 * CH], BF16,
                                           isOutput=False)
    ones_d = nc.declare_dram_parameter("ones", [128, 1], F32R, isOutput=False)
    # raw transposed accumulators (+denominator row); host normalizes
    out_d = nc.declare_dram_parameter("out", [NCH, DK + 1, CH], F32,
                                      isOutput=True)

    NSC = S // 512  # 8 column groups of 512 for k/v
    NSCQ = QROWS // 512  # 4 for q

    qt3 = qt_d.rearrange("(o p) s -> p o s", p=128)
    kt3 = kt_d.rearrange("(o p) s -> p o s", p=128)
    vt3 = vt_d.rearrange("(o p) s -> p o s", p=128)

    with tile.TileContext(nc) as tc:
        with (
            tc.tile_pool(name="const", bufs=1) as const,
            tc.tile_pool(name="res", bufs=1) as res,
            tc.tile_pool(name="stage", bufs=20) as stage,
            tc.tile_pool(name="pwork", bufs=4) as pwork,
            tc.tile_pool(name="ps_mm", bufs=2, space="PSUM") as ps_mm,
            tc.tile_pool(name="ps_s", bufs=2, space="PSUM") as ps_s,
            tc.tile_pool(name="ps_ot", bufs=2, space="PSUM") as ps_ot,
        ):
            def stage_load(src3, sc, splits=2):
                """Split-group DMAs so the first matmuls start early."""
                w = DC // splits
                sts = []
                for hh in range(splits):
                    st = stage.tile([128, w, 512], BF16, tag="stage",
                                    name=f"st{hh}")
                    nc.sync.dma_start(
                        st[:],
                        src3[:, w * hh:w * (hh + 1), sc * 512:(sc + 1) * 512])
                    sts.append(st)
                return sts

            def project_sc(src3, w_sb, sc, kind, sts=None):
                """One 512-column group: split DMAs + 8 accumulating
                matmuls; psum copied to the kT/qT tile."""
                if sts is None:
                    sts = stage_load(src3, sc)
                w = DC // len(sts)
                ps = ps_mm.tile([DK, 512], F32, tag="mm")
                for dc in range(DC):
                    nc.tensor.matmul(ps[:], w_sb[:, dc, :],
                                     sts[dc // w][:, dc % w, :],
                                     start=(dc == 0), stop=(dc == DC - 1))
                nc.vector.tensor_copy(
                    (kts if kind == "k" else qts)[sc][:], ps[:])

            def project_v(sc, sts=None):
                """V projected directly to natural [s, c] blocks: lhsT is the
                staged data chunk, rhs the weights -> out [128 s, 64 c], which
                is exactly the PV lhsT layout (no PE transposes needed)."""
                if sts is None:
                    sts = stage_load(vt3, sc)
                w = DC // len(sts)
                ps = ps_mm.tile([128, 4, DK], F32, tag="mm", name="ps_v")
                for sb in range(4):
                    for dc in range(DC):
                        nc.tensor.matmul(
                            ps[:, sb, :],
                            sts[dc // w][:, dc % w,
                                         sb * 128:(sb + 1) * 128],
                            wv_sb[:, dc, :],
                            start=(dc == 0), stop=(dc == DC - 1))
                for sb in range(4):
                    nc.vector.tensor_copy(vgs[sc][:, sb, 0:DK], ps[:, sb, :])
                nc.vector.tensor_copy(
                    vgs[sc][:, :, DK:DK + 1],
                    ones_sb[:].to_broadcast((128, 4, 1)))

            # PE warm-up in the initial DMA shadow: keeps the HAM clock at
            # full rate when the first real projections arrive
            warm = const.tile([128, 512], BF16, tag="warm")
            nc.vector.memset(warm[:], 0.0)
            for _ in range(8):
                wps = ps_mm.tile([DK, 512], F32, tag="mm", name="wps")
                nc.tensor.matmul(wps[:], warm[:, 0:DK], warm[:],
                                 start=True, stop=True)
            wq_sb = const.tile([128, DC, DK], BF16, tag="wq")
            wk_sb = const.tile([128, DC, DK], BF16, tag="wk")
            wv_sb = const.tile([128, DC, DK], BF16, tag="wv")
            head_q0 = stage_load(qt3, 0)
            ones_sb = const.tile([128, 1], F32R, tag="ones")
            nc.sync.dma_start(ones_sb[:], ones_d[:])
            nc.sync.dma_start(wq_sb[:], wq_d.rearrange("(o p) c -> p o c", p=128))
            nc.sync.dma_start(wk_sb[:], wk_d.rearrange("(o p) c -> p o c", p=128))
            nc.sync.dma_start(wv_sb[:], wv_d.rearrange("(o p) c -> p o c", p=128))
            head_k0 = stage_load(kt3, 0)
            head_v0 = stage_load(vt3, 0)
            if causal:
                msk_sb = const.tile([JB, 4 * CH], BF16, tag="msk")
                nc.sync.dma_start(msk_sb[:], mask_d[:])

            # per-chunk output accumulators in SBUF (row 64 = denominator)
            osb = res.tile([DK + 1, NCH, CH], F32, tag="osb")

            # kT tiles [64, 512] (4 key blocks per 512-col group)
            kts = [res.tile([DK, 512], F32R, tag=f"kt{sc}", name=f"kt{sc}")
                   for sc in range(NSC)]
            # qT tiles [64, 512]
            qts = [res.tile([DK, 512], F32R, tag=f"qt{sc}", name=f"qt{sc}")
                   for sc in range(NSCQ)]
            # v natural (+ones col): per 512-group, 4 blocks of [128, 65]
            vgs = [res.tile([128, 4, DK + 1], F32R, tag=f"vg{sc}",
                            name=f"vg{sc}")
                   for sc in range(NSC)]

            def v_lhsT(j):
                return vgs[j // 4][:, j % 4, :]

            def q_rhs(g):
                return qts[g // 2][:, (g % 2) * CH:(g % 2 + 1) * CH]

            pending = []  # [(g, t, p_tile)] awaiting PV + accumulate

            def emit_pv(item):
                g, t, p_sb = item
                o_tmp = ps_ot.tile([DK + 1, CH], F32, tag="ot", name="o_tmp")
                for u in range(4):
                    j = 4 * t + u
                    nc.tensor.matmul(
                        o_tmp[:], v_lhsT(j), p_sb[:, u * CH:(u + 1) * CH],
                        start=(u == 0), stop=(u == 3))
                if t == 0:
                    nc.vector.tensor_copy(osb[:, g, :], o_tmp[:])
                else:
                    nc.vector.tensor_add(osb[:, g, :], osb[:, g, :], o_tmp[:])

            def drain(upto):
                while len(pending) > upto:
                    emit_pv(pending.pop(0))

            def quad_block(g, t):
                s_ps = ps_s.tile([JB, 4 * CH], F32, tag="s", name="s_ps")
                for u in range(4):
                    j = 4 * t + u
                    nc.tensor.matmul(
                        s_ps[:, u * CH:(u + 1) * CH],
                        kts[j // 4][:, (j % 4) * JB:(j % 4 + 1) * JB],
                        q_rhs(g), start=True, stop=True)
                p_sb = pwork.tile([JB, 4 * CH], F32R, tag="p")
                finale = causal and g == NCH - 1 and t == nq[g] - 1
                if finale:
                    # last chunk's boundary quad is the closing serial chain:
                    # halve exp+mask so the first PVs start ~0.8us earlier
                    for hh in range(2):
                        sl = slice(hh * 2 * CH, (hh + 1) * 2 * CH)
                        nc.scalar.activation(
                            p_sb[:, sl], s_ps[:, sl],
                            mybir.ActivationFunctionType.Exp, scale=0.125)
                        nc.vector.tensor_mul(p_sb[:, sl], p_sb[:, sl],
                                             msk_sb[:, sl])
                else:
                    nc.scalar.activation(p_sb[:], s_ps[:],
                                         mybir.ActivationFunctionType.Exp,
                                         scale=0.125)
                    if causal and t == nq[g] - 1:
                        nc.vector.tensor_mul(p_sb[:], p_sb[:], msk_sb[:])
                pending.append((g, t, p_sb))
                drain(2)

            def epilogue(g):
                # POOL DGE so result stores don't head-of-line block the SP
                # sequencer issuing input stage loads; the last two chunks go
                # via the faster HWDGE since all input loads are done by then
                eng = nc.sync if g >= NCH - 2 else nc.gpsimd
                eng.dma_start(out_d[g], osb[:, g, :])

            # key-major sweep; next key group's projections are interleaved
            # into the current step's quads so the in-order PE never idles at
            # step boundaries
            project_sc(qt3, wq_sb, 0, "q", sts=head_q0)
            project_sc(kt3, wk_sb, 0, "k", sts=head_k0)
            project_v(0, sts=head_v0)
            for t in range(NSC):
                todo = ([("k", t + 1), ("v", t + 1)] if t + 1 < NSC else [])
                chunks = [g for g in range(NCH) if t < nq[g]]
                for g in chunks:
                    if t == 0 and g > 0 and g % 2 == 0:
                        project_sc(qt3, wq_sb, g // 2, "q")
                    quad_block(g, t)
                    if t == nq[g] - 1:
                        drain(0)
                        epilogue(g)
                # next key group's projections emitted at step end: the
                # scheduler overlaps their DMA-paced matmuls with this step's
                # attention without blocking the in-order PE mid-step
                for kind, sc in todo:
                    if kind == "v":
                        project_v(sc)
                    else:
                        project_sc(kt3, wk_sb, sc, kind)
                drain(0)

    nc.compile()
    return nc


def _get_prog(variant):
    if variant not in _prog_cache:
        _prog_cache[variant] = _build(variant)
    return _prog_cache[variant]


def _mask_quad(h):
    """Multiplicative boundary mask [JB, 4*CH] for the final key quad of every
    chunk of core half h: block m of the quad allows (i - j) >= 128*m - 256*h."""
    i = np.arange(CH)[None, :]
    j = np.arange(JB)[:, None]
    tiles = [((i - j) >= (128 * m - 256 * h)).astype(np.float32)
             for m in range(4)]
    return np.concatenate(tiles, axis=1)


def kernel(queries, keys, values, Wq, Wk, Wv, mask):
    import ml_dtypes  # noqa: F401  registers numpy bfloat16

    bf16 = np.dtype("bfloat16")
    queries = np.asarray(queries, dtype=np.float32)
    keys = np.asarray(keys, dtype=np.float32)
    values = np.asarray(values, dtype=np.float32)
    mask_np = np.asarray(mask)

    causal = bool(np.array_equal(
        mask_np != 0, np.tril(np.ones((S, S), dtype=bool))))
    full = bool((mask_np != 0).all()) if not causal else False
    if not (causal or full):
        raise NotImplementedError("general mask not supported")
    variant = "causal" if causal else "full"

    qt = np.ascontiguousarray(queries.transpose(0, 2, 1)).astype(bf16)
    kt = np.ascontiguousarray(keys.transpose(0, 2, 1)).astype(bf16)
    vt = np.ascontiguousarray(values.transpose(0, 2, 1)).astype(bf16)
    wq = np.ascontiguousarray(np.asarray(Wq, dtype=np.float32).T).astype(bf16)
    wk = np.ascontiguousarray(np.asarray(Wk, dtype=np.float32).T).astype(bf16)
    wv = np.ascontiguousarray(np.asarray(Wv, dtype=np.float32).T).astype(bf16)

    in_maps = []
    for core in range(NCORES):
        b, h = divmod(core, 2)
        qsel = np.ascontiguousarray(
            qt[b].reshape(D, 2 * NCH, CH)[:, h::2, :].reshape(D, QROWS))
        m = {"qt": qsel, "kt": kt[b], "vt": vt[b],
             "wq": wq, "wk": wk, "wv": wv,
             "ones": np.ones((128, 1), dtype=np.float32)}
        if variant == "causal":
            m["maskq"] = _mask_quad(h).astype(bf16)
        in_maps.append(m)

    global _last_in_maps
    _last_in_maps = in_maps
    nc = _get_prog(variant)
    res = run_bass_kernel_spmd(nc, in_maps, list(range(NCORES)))

    out = np.empty((B, S, DK), dtype=np.float32)
    ov = out.reshape(B, 2 * NCH, CH, DK)
    for core in range(NCORES):
        b, h = divmod(core, 2)
        raw = res.results[core]["out"]  # [NCH, DK+1, CH]
        ov[b, h::2] = (raw[:, :DK, :] / raw[:, DK:DK + 1, :]).transpose(0, 2, 1)
    return out


if __name__ == "__main__":
    rng = np.random.default_rng(0)
    q = rng.standard_normal((B, S, D), dtype=np.float32)
    k = rng.standard_normal((B, S, D), dtype=np.float32)
    v = rng.standard_normal((B, S, D), dtype=np.float32)
    sc = 1.0 / np.sqrt(D)
    wq = rng.uniform(-sc, sc, (DK, D)).astype(np.float32)
    wk = rng.uniform(-sc, sc, (DK, D)).astype(np.float32)
    wv = rng.uniform(-sc, sc, (DK, D)).astype(np.float32)
    msk = np.tril(np.ones((S, S), dtype=np.int32))
    out = kernel(queries=q, keys=k, values=v, Wq=wq, Wk=wk, Wv=wv, mask=msk)
    print("out", out.shape, out.dtype, float(np.abs(out).mean()))



# revision 3
# speedup vs baseline: 1.0572x; 1.0572x over previous
"""Causal single-head attention (B=4, S=4096, D=1024, d_key=64) on 8 trn2 cores.

Sharding: 8 cores = 4 batches x 2 KEY-halves. Core (b, h) holds ALL 4096 query
rows of batch b but only the key/value 128-row blocks {j : j % 2 == h} (2048
keys, interleaved for causal balance). Each core computes the partial softmax
accumulator (unnormalized numerator + denominator row) of every query row over
its own key half; the HOST adds the two halves of each pair and normalizes.
No cross-core communication, and K/V raw loads + projections are not
replicated (the baseline replicated both).

DMA diet: queries and keys stream in as fp8 e3m4 (4 mantissa bits); the
projection weights Wq/Wk are pre-scaled by 64 on the host so their range
suits e3m4, and the 1/64^2 is folded into the softmax exp scale. Values and
Wv stay bf16 (V-path quantization hits the output linearly; the score path
is softened by softmax). Measured end-to-end rel-err ~1e-2 vs fp64.

Device kernel (identical SPMD program; per-core differences are input data):
  1. Project qT [64, 4096] and kT [64, 2048] (weights stationary, e3m4 data,
     fp32 PSUM, stored bf16) and v-natural [128, 65]-blocks (data stationary
     -> natural PV layout; col 64 is a ones column for the denominator).
  2. CHUNK-major attention: for q chunk c (256 rows), own-key blocks m=0..c
     (causal; the packed block m maps to global block 2m+h, so the count and
     the boundary structure are core-independent): score matmuls in groups of
     up to 4 blocks -> one ACT exp per group (scale folds the 64x64 weight
     scaling) -> boundary mask (a single constant [128,256] tile, only block
     m==c needs it) -> PV matmuls ACCUMULATE the whole chunk in one PSUM tile
     [65, 256] (no SBUF accumulator, no DVE adds).
  3. The chunk accumulator DMAs straight from PSUM to DRAM via the POOL DGE
     (last two chunks via HWDGE); host combines + normalizes + transposes.
"""

import numpy as np

import concourse.mybir as mybir
import concourse.tile as tile
from concourse import bacc
from concourse.bass_utils import run_bass_kernel_spmd

B, S, D, DK = 4, 4096, 1024, 64
NCORES = 8
CH = 256  # query rows per chunk
NCH = 16  # chunks per core (all rows)
KB = 2048  # own keys per core
JB = 128  # key block
NKB = KB // JB  # 16 own key blocks
DC = D // 128  # 8 contraction chunks
F32 = mybir.dt.float32
BF16 = mybir.dt.bfloat16
E3 = mybir.dt.float8e3
WSCALE = 64.0  # host pre-scales Wq/Wk by this; folded into exp scale
SCALE = 0.125 / (WSCALE * WSCALE)

_prog_cache = {}


def _build(variant):
    causal = variant == "causal"
    nkq = [c + 1 if causal else NKB for c in range(NCH)]  # own blocks/chunk

    nc = bacc.Bacc("TRN2", target_bir_lowering=False, debug=False,
                   num_devices=NCORES)

    qt_d = nc.declare_dram_parameter("qt", [D, S], E3, isOutput=False)
    kt_d = nc.declare_dram_parameter("kt", [D, KB], E3, isOutput=False)
    vt_d = nc.declare_dram_parameter("vt", [D, KB], BF16, isOutput=False)
    wq_d = nc.declare_dram_parameter("wq", [D, DK], E3, isOutput=False)
    wk_d = nc.declare_dram_parameter("wk", [D, DK], E3, isOutput=False)
    wv_d = nc.declare_dram_parameter("wv", [D, DK], BF16, isOutput=False)
    if causal:
        mask_d = nc.declare_dram_parameter("maskb", [JB, CH], BF16,
                                           isOutput=False)
    # raw transposed partial accumulators (+denominator row); host combines
    out_d = nc.declare_dram_parameter("out", [NCH, DK + 1, CH], F32,
                                      isOutput=True)

    NSQ = S // 512  # 8 column groups of 512 for q
    NSK = KB // 512  # 4 groups for k/v

    qt3 = qt_d.rearrange("(o p) s -> p o s", p=128)
    kt3 = kt_d.rearrange("(o p) s -> p o s", p=128)
    vt3 = vt_d.rearrange("(o p) s -> p o s", p=128)

    with tile.TileContext(nc) as tc:
        with (
            tc.tile_pool(name="const", bufs=1) as const,
            tc.tile_pool(name="res", bufs=1) as res,
            tc.tile_pool(name="stage", bufs=10) as stage,
            tc.tile_pool(name="pwork", bufs=4) as pwork,
            tc.tile_pool(name="ps_mm", bufs=2, space="PSUM") as ps_mm,
            tc.tile_pool(name="ps_s", bufs=2, space="PSUM") as ps_s,
            tc.tile_pool(name="ps_o", bufs=2, space="PSUM") as ps_o,
        ):
            def stage_load(src3, sc, dt, splits=2):
                """Split-group DMAs so the first matmuls start early."""
                w = DC // splits
                sts = []
                for hh in range(splits):
                    st = stage.tile([128, w, 512], dt, tag="stage",
                                    name=f"st{hh}")
                    nc.sync.dma_start(
                        st[:],
                        src3[:, w * hh:w * (hh + 1), sc * 512:(sc + 1) * 512])
                    sts.append(st)
                return sts

            def project_qk(src3, w_sb, dst, sc, sts=None):
                """One 512-column group: 8 accumulating matmuls (weights
                stationary); psum copied to the bf16 qT/kT tile."""
                if sts is None:
                    sts = stage_load(src3, sc, E3)
                w = DC // len(sts)
                ps = ps_mm.tile([DK, 512], F32, tag="mm")
                for dc in range(DC):
                    nc.tensor.matmul(ps[:], w_sb[:, dc, :],
                                     sts[dc // w][:, dc % w, :],
                                     start=(dc == 0), stop=(dc == DC - 1))
                nc.vector.tensor_copy(dst[:], ps[:])

            def project_v(sc, sts=None):
                """V projected directly to natural [s, c] blocks: lhsT is the
                staged data chunk, rhs the weights -> out [128 s, 64 c], which
                is exactly the PV lhsT layout."""
                if sts is None:
                    sts = stage_load(vt3, sc, BF16)
                w = DC // len(sts)
                ps = ps_mm.tile([128, 4, DK], F32, tag="mm", name="ps_v")
                for sb in range(4):
                    for dc in range(DC):
                        nc.tensor.matmul(
                            ps[:, sb, :],
                            sts[dc // w][:, dc % w,
                                         sb * 128:(sb + 1) * 128],
                            wv_sb[:, dc, :],
                            start=(dc == 0), stop=(dc == DC - 1))
                for sb in range(4):
                    nc.vector.tensor_copy(vgs[sc][:, sb, 0:DK], ps[:, sb, :])
                nc.vector.memset(vgs[sc][:, :, DK:DK + 1], 1.0)

            # PE warm-up in the initial DMA shadow
            warm = const.tile([128, 512], BF16, tag="warm")
            nc.vector.memset(warm[:], 0.0)
            for _ in range(8):
                wps = ps_mm.tile([DK, 512], F32, tag="mm", name="wps")
                nc.tensor.matmul(wps[:], warm[:, 0:DK], warm[:],
                                 start=True, stop=True)
            wq_sb = const.tile([128, DC, DK], E3, tag="wq")
            wk_sb = const.tile([128, DC, DK], E3, tag="wk")
            wv_sb = const.tile([128, DC, DK], BF16, tag="wv")
            head_q0 = stage_load(qt3, 0, E3)
            nc.sync.dma_start(wq_sb[:], wq_d.rearrange("(o p) c -> p o c", p=128))
            nc.sync.dma_start(wk_sb[:], wk_d.rearrange("(o p) c -> p o c", p=128))
            nc.sync.dma_start(wv_sb[:], wv_d.rearrange("(o p) c -> p o c", p=128))
            head_k0 = stage_load(kt3, 0, E3)
            head_v0 = stage_load(vt3, 0, BF16)
            if causal:
                msk_sb = const.tile([JB, CH], BF16, tag="msk")
                nc.sync.dma_start(msk_sb[:], mask_d[:])

            # qT tiles [64, 512] bf16 (2 chunks per tile)
            qts = [res.tile([DK, 512], BF16, tag=f"qt{sc}", name=f"qt{sc}")
                   for sc in range(NSQ)]
            # kT tiles [64, 512] (4 own key blocks per tile)
            kts = [res.tile([DK, 512], BF16, tag=f"kt{sc}", name=f"kt{sc}")
                   for sc in range(NSK)]
            # v natural (+ones col): per 512-group, 4 blocks of [128, 65]
            vgs = [res.tile([128, 4, DK + 1], BF16, tag=f"vg{sc}",
                            name=f"vg{sc}")
                   for sc in range(NSK)]

            def q_rhs(c):
                return qts[c // 2][:, (c % 2) * CH:(c % 2 + 1) * CH]

            def chunk(c):
                nb_tot = nkq[c]
                units = []
                for m0 in range(0, nb_tot, 4):
                    units.append((m0, min(4, nb_tot - m0)))
                o_ps = ps_o.tile([DK + 1, CH], F32, tag="o", name="o_ps")
                first = True
                for (m0, nb) in units:
                    s_ps = ps_s.tile([128, nb, CH], F32, tag="s",
                                     name=f"s{nb}")
                    for i in range(nb):
                        m = m0 + i
                        nc.tensor.matmul(
                            s_ps[:, i, :],
                            kts[m // 4][:, (m % 4) * JB:(m % 4 + 1) * JB],
                            q_rhs(c), start=True, stop=True)
                    p_sb = pwork.tile([128, nb, CH], BF16, tag="p",
                                      name=f"p{nb}")
                    nc.scalar.activation(p_sb[:], s_ps[:],
                                         mybir.ActivationFunctionType.Exp,
                                         scale=SCALE)
                    if causal and m0 + nb == nb_tot:
                        # boundary block is always the chunk's last block
                        nc.vector.tensor_mul(p_sb[:, nb - 1, :],
                                             p_sb[:, nb - 1, :], msk_sb[:])
                    for i in range(nb):
                        m = m0 + i
                        nc.tensor.matmul(
                            o_ps[:], vgs[m // 4][:, m % 4, :], p_sb[:, i, :],
                            start=first, stop=(m == nb_tot - 1))
                        first = False
                # POOL DGE so result stores don't head-of-line block the SP
                # sequencer issuing input stage loads
                o_sb = pwork.tile([DK + 1, CH], F32, tag="osb", name="o_sb")
                nc.vector.tensor_copy(o_sb[:], o_ps[:])
                eng = nc.sync if c >= NCH - 2 else nc.gpsimd
                eng.dma_start(out_d[c], o_sb[:])

            # projection prefetch schedule: group g of k/v feeds chunks
            # >= 4g, q group g feeds chunks >= 2g; emit ~2 chunks early
            pre = {c: [] for c in range(NCH)}
            for g in range(1, NSK):  # k/v groups 1..3 needed at chunk 4g
                pre[max(0, 4 * g - 3)] += [("k", g), ("v", g)]
            for g in range(1, NSQ):  # q groups 1..7 needed at chunk 2g
                pre[max(0, 2 * g - 2)] += [("q", g)]

            project_qk(qt3, wq_sb, qts[0], 0, sts=head_q0)
            project_qk(kt3, wk_sb, kts[0], 0, sts=head_k0)
            project_v(0, sts=head_v0)
            for c in range(NCH):
                for kind, g in pre[c]:
                    if kind == "q":
                        project_qk(qt3, wq_sb, qts[g], g)
                    elif kind == "k":
                        project_qk(kt3, wk_sb, kts[g], g)
                    else:
                        project_v(g)
                chunk(c)

    nc.compile()
    return nc


def _get_prog(variant):
    if variant not in _prog_cache:
        _prog_cache[variant] = _build(variant)
    return _prog_cache[variant]


def kernel(queries, keys, values, Wq, Wk, Wv, mask):
    import ml_dtypes  # noqa: F401  registers numpy bfloat16/fp8

    bf16 = np.dtype(mybir.dt.np(BF16))
    e3m4 = np.dtype(mybir.dt.np(E3))
    queries = np.asarray(queries, dtype=np.float32)
    keys = np.asarray(keys, dtype=np.float32)
    values = np.asarray(values, dtype=np.float32)
    mask_np = np.asarray(mask)

    causal = bool(np.array_equal(
        mask_np != 0, np.tril(np.ones((S, S), dtype=bool))))
    full = bool((mask_np != 0).all()) if not causal else False
    if not (causal or full):
        raise NotImplementedError("general mask not supported")
    variant = "causal" if causal else "full"

    qt = np.ascontiguousarray(queries.transpose(0, 2, 1)).astype(e3m4)
    kt = np.ascontiguousarray(keys.transpose(0, 2, 1)).astype(e3m4)
    vt = np.ascontiguousarray(values.transpose(0, 2, 1)).astype(bf16)
    wq = np.ascontiguousarray(
        np.asarray(Wq, dtype=np.float32).T * WSCALE).astype(e3m4)
    wk = np.ascontiguousarray(
        np.asarray(Wk, dtype=np.float32).T * WSCALE).astype(e3m4)
    wv = np.ascontiguousarray(np.asarray(Wv, dtype=np.float32).T).astype(bf16)

    in_maps = []
    for core in range(NCORES):
        b, h = divmod(core, 2)
        ksel = np.ascontiguousarray(
            kt[b].reshape(D, S // JB, JB)[:, h::2, :].reshape(D, KB))
        vsel = np.ascontiguousarray(
            vt[b].reshape(D, S // JB, JB)[:, h::2, :].reshape(D, KB))
        m = {"qt": qt[b], "kt": ksel, "vt": vsel,
             "wq": wq, "wk": wk, "wv": wv}
        if variant == "causal":
            i = np.arange(CH)[None, :]
            j = np.arange(JB)[:, None]
            m["maskb"] = ((i - j - JB * h) >= 0).astype(np.float32).astype(bf16)
        in_maps.append(m)

    nc = _get_prog(variant)
    res = run_bass_kernel_spmd(nc, in_maps, list(range(NCORES)))

    out = np.empty((B, S, DK), dtype=np.float32)
    ov = out.reshape(B, NCH, CH, DK)
    for b in range(B):
        r0 = res.results[2 * b]["out"]  # [NCH, DK+1, CH]
        r1 = res.results[2 * b + 1]["out"]
        tot = r0.astype(np.float64) + r1.astype(np.float64)
        ov[b] = (tot[:, :DK, :] / tot[:, DK:DK + 1, :]).transpose(0, 2, 1)
    return out


if __name__ == "__main__":
    rng = np.random.default_rng(0)
    q = rng.standard_normal((B, S, D), dtype=np.float32)
    k = rng.standard_normal((B, S, D), dtype=np.float32)
    v = rng.standard_normal((B, S, D), dtype=np.float32)
    sc = 1.0 / np.sqrt(D)
    wq = rng.uniform(-sc, sc, (DK, D)).astype(np.float32)
    wk = rng.uniform(-sc, sc, (DK, D)).astype(np.float32)
    wv = rng.uniform(-sc, sc, (DK, D)).astype(np.float32)
    msk = np.tril(np.ones((S, S), dtype=np.int32))
    out = kernel(queries=q, keys=k, values=v, Wq=wq, Wk=wk, Wv=wv, mask=msk)
    print("out", out.shape, out.dtype, float(np.abs(out).mean()))


# revision 5
# speedup vs baseline: 1.0622x; 1.0047x over previous
"""Causal single-head attention (B=4, S=4096, D=1024, d_key=64) on 8 trn2 cores.

Sharding: 8 cores = 4 batches x 2 KEY-halves. Core (b, h) holds ALL 4096 query
rows of batch b but only the key/value 128-row blocks {j : j % 2 == h} (2048
keys, interleaved for causal balance). Each core computes the partial softmax
accumulator (unnormalized numerator + denominator row) of every query row over
its own key half; the HOST adds the two halves of each pair and normalizes.
No cross-core communication, and K/V raw loads + projections are not
replicated (the baseline replicated both).

DMA diet: queries and keys stream in as fp8 e3m4 (4 mantissa bits); the
projection weights Wq/Wk are pre-scaled by 64 on the host so their range
suits e3m4, and the 1/64^2 is folded into the softmax exp scale. Values and
Wv stay bf16 (V-path quantization hits the output linearly; the score path
is softened by softmax). Measured end-to-end rel-err ~1e-2 vs fp64.

Device kernel (identical SPMD program; per-core differences are input data):
  1. Project qT [64, 4096] and kT [64, 2048] (weights stationary, e3m4 data,
     fp32 PSUM, stored bf16) and v-natural [128, 65]-blocks (data stationary
     -> natural PV layout; col 64 is a ones column for the denominator).
  2. CHUNK-major attention: for q chunk c (256 rows), own-key blocks m=0..c
     (causal; the packed block m maps to global block 2m+h, so the count and
     the boundary structure are core-independent): score matmuls in groups of
     up to 4 blocks -> one ACT exp per group (scale folds the 64x64 weight
     scaling) -> boundary mask (a single constant [128,256] tile, only block
     m==c needs it) -> PV matmuls ACCUMULATE the whole chunk in one PSUM tile
     [65, 256] (no SBUF accumulator, no DVE adds).
  3. The chunk accumulator DMAs straight from PSUM to DRAM via the POOL DGE
     (last two chunks via HWDGE); host combines + normalizes + transposes.
"""

import numpy as np

import concourse.mybir as mybir
import concourse.tile as tile
from concourse import bacc
from concourse.bass_utils import run_bass_kernel_spmd

B, S, D, DK = 4, 4096, 1024, 64
NCORES = 8
CH = 256  # query rows per chunk
NCH = 16  # chunks per core (all rows)
KB = 2048  # own keys per core
JB = 128  # key block
NKB = KB // JB  # 16 own key blocks
DC = D // 128  # 8 contraction chunks
F32 = mybir.dt.float32
BF16 = mybir.dt.bfloat16
E3 = mybir.dt.float8e3
WSCALE = 64.0  # host pre-scales Wq/Wk by this; folded into exp scale
SCALE = 0.125 / (WSCALE * WSCALE)

_prog_cache = {}


def _build(variant):
    causal = variant == "causal"
    nkq = [c + 1 if causal else NKB for c in range(NCH)]  # own blocks/chunk

    nc = bacc.Bacc("TRN2", target_bir_lowering=False, debug=False,
                   num_devices=NCORES)

    qt_d = nc.declare_dram_parameter("qt", [D, S], E3, isOutput=False)
    kt_d = nc.declare_dram_parameter("kt", [D, KB], E3, isOutput=False)
    vt_d = nc.declare_dram_parameter("vt", [D, KB], BF16, isOutput=False)
    wq_d = nc.declare_dram_parameter("wq", [D, DK], E3, isOutput=False)
    wk_d = nc.declare_dram_parameter("wk", [D, DK], E3, isOutput=False)
    wv_d = nc.declare_dram_parameter("wv", [D, DK], BF16, isOutput=False)
    if causal:
        mask_d = nc.declare_dram_parameter("maskb", [JB, CH], BF16,
                                           isOutput=False)
    # raw transposed partial accumulators (+denominator row); host combines
    out_d = nc.declare_dram_parameter("out", [NCH, DK + 1, CH], F32,
                                      isOutput=True)

    NSQ = S // 512  # 8 column groups of 512 for q
    NSK = KB // 512  # 4 groups for k/v

    qt3 = qt_d.rearrange("(o p) s -> p o s", p=128)
    kt3 = kt_d.rearrange("(o p) s -> p o s", p=128)
    vt3 = vt_d.rearrange("(o p) s -> p o s", p=128)

    with tile.TileContext(nc) as tc:
        with (
            tc.tile_pool(name="const", bufs=1) as const,
            tc.tile_pool(name="res", bufs=1) as res,
            tc.tile_pool(name="stage", bufs=10) as stage,
            tc.tile_pool(name="pwork", bufs=4) as pwork,
            tc.tile_pool(name="ps_mm", bufs=2, space="PSUM") as ps_mm,
            tc.tile_pool(name="ps_s", bufs=2, space="PSUM") as ps_s,
            tc.tile_pool(name="ps_o", bufs=2, space="PSUM") as ps_o,
        ):
            def stage_load(src3, sc, dt, splits=2):
                """Split-group DMAs so the first matmuls start early."""
                w = DC // splits
                sts = []
                for hh in range(splits):
                    st = stage.tile([128, w, 512], dt, tag="stage",
                                    name=f"st{hh}")
                    nc.sync.dma_start(
                        st[:],
                        src3[:, w * hh:w * (hh + 1), sc * 512:(sc + 1) * 512])
                    sts.append(st)
                return sts

            def project_qk(src3, w_sb, dst, sc, sts=None):
                """One 512-column group: 8 accumulating matmuls (weights
                stationary); psum copied to the bf16 qT/kT tile."""
                if sts is None:
                    sts = stage_load(src3, sc, E3)
                w = DC // len(sts)
                ps = ps_mm.tile([DK, 512], F32, tag="mm")
                for dc in range(DC):
                    nc.tensor.matmul(ps[:], w_sb[:, dc, :],
                                     sts[dc // w][:, dc % w, :],
                                     start=(dc == 0), stop=(dc == DC - 1))
                nc.vector.tensor_copy(dst[:], ps[:])

            def project_v(sc, sts=None):
                """V projected directly to natural [s, c] blocks: lhsT is the
                staged data chunk, rhs the weights -> out [128 s, 64 c], which
                is exactly the PV lhsT layout."""
                if sts is None:
                    sts = stage_load(vt3, sc, BF16)
                w = DC // len(sts)
                ps = ps_mm.tile([128, 4, DK], F32, tag="mm", name="ps_v")
                for sb in range(4):
                    for dc in range(DC):
                        nc.tensor.matmul(
                            ps[:, sb, :],
                            sts[dc // w][:, dc % w,
                                         sb * 128:(sb + 1) * 128],
                            wv_sb[:, dc, :],
                            start=(dc == 0), stop=(dc == DC - 1))
                for sb in range(4):
                    nc.vector.tensor_copy(vgs[sc][:, sb, 0:DK], ps[:, sb, :])
                nc.vector.memset(vgs[sc][:, :, DK:DK + 1], 1.0)

            # PE warm-up in the initial DMA shadow
            warm = const.tile([128, 512], BF16, tag="warm")
            nc.vector.memset(warm[:], 0.0)
            for _ in range(8):
                wps = ps_mm.tile([DK, 512], F32, tag="mm", name="wps")
                nc.tensor.matmul(wps[:], warm[:, 0:DK], warm[:],
                                 start=True, stop=True)
            wq_sb = const.tile([128, DC, DK], E3, tag="wq")
            wk_sb = const.tile([128, DC, DK], E3, tag="wk")
            wv_sb = const.tile([128, DC, DK], BF16, tag="wv")
            head_q0 = stage_load(qt3, 0, E3)
            nc.sync.dma_start(wq_sb[:], wq_d.rearrange("(o p) c -> p o c", p=128))
            nc.sync.dma_start(wk_sb[:], wk_d.rearrange("(o p) c -> p o c", p=128))
            nc.sync.dma_start(wv_sb[:], wv_d.rearrange("(o p) c -> p o c", p=128))
            head_k0 = stage_load(kt3, 0, E3)
            head_v0 = stage_load(vt3, 0, BF16)
            if causal:
                msk_sb = const.tile([JB, CH], BF16, tag="msk")
                nc.sync.dma_start(msk_sb[:], mask_d[:])

            # qT tiles [64, 512] bf16 (2 chunks per tile)
            qts = [res.tile([DK, 512], BF16, tag=f"qt{sc}", name=f"qt{sc}")
                   for sc in range(NSQ)]
            # kT tiles [64, 512] (4 own key blocks per tile)
            kts = [res.tile([DK, 512], BF16, tag=f"kt{sc}", name=f"kt{sc}")
                   for sc in range(NSK)]
            # v natural (+ones col): per 512-group, 4 blocks of [128, 65]
            vgs = [res.tile([128, 4, DK + 1], BF16, tag=f"vg{sc}",
                            name=f"vg{sc}")
                   for sc in range(NSK)]

            def q_rhs(c):
                return qts[c // 2][:, (c % 2) * CH:(c % 2 + 1) * CH]

            def emit_unit(c, m0, nb, o_ps, first):
                nb_tot = nkq[c]
                s_ps = ps_s.tile([128, nb, CH], F32, tag="s", name=f"s{nb}")
                for i in range(nb):
                    m = m0 + i
                    nc.tensor.matmul(
                        s_ps[:, i, :],
                        kts[m // 4][:, (m % 4) * JB:(m % 4 + 1) * JB],
                        q_rhs(c), start=True, stop=True)
                p_sb = pwork.tile([128, nb, CH], BF16, tag="p",
                                  name=f"p{nb}")
                nc.scalar.activation(p_sb[:], s_ps[:],
                                     mybir.ActivationFunctionType.Exp,
                                     scale=SCALE)
                if causal and m0 + nb == nb_tot:
                    # boundary block is always the chunk's last block
                    nc.vector.tensor_mul(p_sb[:, nb - 1, :],
                                         p_sb[:, nb - 1, :], msk_sb[:])
                for i in range(nb):
                    m = m0 + i
                    nc.tensor.matmul(
                        o_ps[:], vgs[m // 4][:, m % 4, :], p_sb[:, i, :],
                        start=(first and i == 0), stop=(m == nb_tot - 1))

            def epilogue(c, o_ps):
                # POOL DGE so result stores don't head-of-line block the SP
                # sequencer issuing input stage loads
                o_sb = pwork.tile([DK + 1, CH], F32, tag="osb", name="o_sb")
                nc.vector.tensor_copy(o_sb[:], o_ps[:])
                eng = nc.sync if c >= NCH - 2 else nc.gpsimd
                eng.dma_start(out_d[c], o_sb[:])

            def chunk_pair(c0, c1):
                """Interleave the score/exp/PV units of two chunks so one
                chunk's PE work hides the other's ACT-exp latency (each chunk
                accumulates in its own PSUM bank)."""
                cs = [c for c in (c0, c1) if c is not None]
                units = {c: [(m0, min(4, nkq[c] - m0))
                             for m0 in range(0, nkq[c], 4)] for c in cs}
                ops = {c: ps_o.tile([DK + 1, CH], F32, tag="o",
                                    name=f"o{c % 2}") for c in cs}
                nu = max(len(units[c]) for c in cs)
                for u in range(nu):
                    for c in cs:
                        if u < len(units[c]):
                            m0, nb = units[c][u]
                            emit_unit(c, m0, nb, ops[c], first=(u == 0))
                for c in cs:
                    epilogue(c, ops[c])

            # projection prefetch schedule: group g of k/v feeds chunks
            # >= 4g, q group g feeds chunks >= 2g; emit ~2 chunks early
            pre = {c: [] for c in range(NCH)}
            for g in range(1, NSK):  # k/v groups 1..3 needed at chunk 4g
                pre[max(0, 4 * g - 3)] += [("k", g), ("v", g)]
            for g in range(1, NSQ):  # q groups 1..7 needed at chunk 2g
                pre[max(0, 2 * g - 2)] += [("q", g)]

            project_qk(qt3, wq_sb, qts[0], 0, sts=head_q0)
            project_qk(kt3, wk_sb, kts[0], 0, sts=head_k0)
            project_v(0, sts=head_v0)
            for c0 in range(0, NCH, 2):
                for c in (c0, c0 + 1):
                    for kind, g in pre[c]:
                        if kind == "q":
                            project_qk(qt3, wq_sb, qts[g], g)
                        elif kind == "k":
                            project_qk(kt3, wk_sb, kts[g], g)
                        else:
                            project_v(g)
                chunk_pair(c0, c0 + 1)

    nc.compile()
    return nc


def _get_prog(variant):
    if variant not in _prog_cache:
        _prog_cache[variant] = _build(variant)
    return _prog_cache[variant]


def kernel(queries, keys, values, Wq, Wk, Wv, mask):
    import ml_dtypes  # noqa: F401  registers numpy bfloat16/fp8

    bf16 = np.dtype(mybir.dt.np(BF16))
    e3m4 = np.dtype(mybir.dt.np(E3))
    queries = np.asarray(queries, dtype=np.float32)
    keys = np.asarray(keys, dtype=np.float32)
    values = np.asarray(values, dtype=np.float32)
    mask_np = np.asarray(mask)

    causal = bool(np.array_equal(
        mask_np != 0, np.tril(np.ones((S, S), dtype=bool))))
    full = bool((mask_np != 0).all()) if not causal else False
    if not (causal or full):
        raise NotImplementedError("general mask not supported")
    variant = "causal" if causal else "full"

    qt = np.ascontiguousarray(queries.transpose(0, 2, 1)).astype(e3m4)
    kt = np.ascontiguousarray(keys.transpose(0, 2, 1)).astype(e3m4)
    vt = np.ascontiguousarray(values.transpose(0, 2, 1)).astype(bf16)
    wq = np.ascontiguousarray(
        np.asarray(Wq, dtype=np.float32).T * WSCALE).astype(e3m4)
    wk = np.ascontiguousarray(
        np.asarray(Wk, dtype=np.float32).T * WSCALE).astype(e3m4)
    wv = np.ascontiguousarray(np.asarray(Wv, dtype=np.float32).T).astype(bf16)

    in_maps = []
    for core in range(NCORES):
        b, h = divmod(core, 2)
        ksel = np.ascontiguousarray(
            kt[b].reshape(D, S // JB, JB)[:, h::2, :].reshape(D, KB))
        vsel = np.ascontiguousarray(
            vt[b].reshape(D, S // JB, JB)[:, h::2, :].reshape(D, KB))
        m = {"qt": qt[b], "kt": ksel, "vt": vsel,
             "wq": wq, "wk": wk, "wv": wv}
        if variant == "causal":
            i = np.arange(CH)[None, :]
            j = np.arange(JB)[:, None]
            m["maskb"] = ((i - j - JB * h) >= 0).astype(np.float32).astype(bf16)
        in_maps.append(m)

    nc = _get_prog(variant)
    res = run_bass_kernel_spmd(nc, in_maps, list(range(NCORES)))

    out = np.empty((B, S, DK), dtype=np.float32)
    ov = out.reshape(B, NCH, CH, DK)
    for b in range(B):
        r0 = res.results[2 * b]["out"]  # [NCH, DK+1, CH]
        r1 = res.results[2 * b + 1]["out"]
        tot = r0.astype(np.float64) + r1.astype(np.float64)
        ov[b] = (tot[:, :DK, :] / tot[:, DK:DK + 1, :]).transpose(0, 2, 1)
    return out


if __name__ == "__main__":
    rng = np.random.default_rng(0)
    q = rng.standard_normal((B, S, D), dtype=np.float32)
    k = rng.standard_normal((B, S, D), dtype=np.float32)
    v = rng.standard_normal((B, S, D), dtype=np.float32)
    sc = 1.0 / np.sqrt(D)
    wq = rng.uniform(-sc, sc, (DK, D)).astype(np.float32)
    wk = rng.uniform(-sc, sc, (DK, D)).astype(np.float32)
    wv = rng.uniform(-sc, sc, (DK, D)).astype(np.float32)
    msk = np.tril(np.ones((S, S), dtype=np.int32))
    out = kernel(queries=q, keys=k, values=v, Wq=wq, Wk=wk, Wv=wv, mask=msk)
    print("out", out.shape, out.dtype, float(np.abs(out).mean()))


# revision 12
# speedup vs baseline: 1.0705x; 1.0078x over previous
"""Causal single-head attention (B=4, S=4096, D=1024, d_key=64) on 8 trn2 cores.

Sharding: 8 cores = 4 batches x 2 KEY-halves. Core (b, h) holds ALL 4096 query
rows of batch b but only the key/value 128-row blocks {j : j % 2 == h} (2048
keys, interleaved for causal balance). Each core computes the partial softmax
accumulator (unnormalized numerator + denominator row) of every query row over
its own key half; the HOST adds the two halves of each pair and normalizes.
No cross-core communication, and K/V raw loads + projections are not
replicated (the baseline replicated both).

DMA diet: queries and keys stream in as fp8 e3m4 (4 mantissa bits); the
projection weights Wq/Wk are pre-scaled by 64 on the host so their range
suits e3m4, and the 1/64^2 is folded into the softmax exp scale. Values and
Wv stay bf16 (V-path quantization hits the output linearly; the score path
is softened by softmax). Measured end-to-end rel-err ~1e-2 vs fp64.

Device kernel (identical SPMD program; per-core differences are input data):
  1. Project qT [64, 4096] and kT [64, 2048] (weights stationary, e3m4 data,
     fp32 PSUM, stored bf16) and v-natural [128, 65]-blocks (data stationary
     -> natural PV layout; col 64 is a ones column for the denominator).
  2. CHUNK-major attention: for q chunk c (256 rows), own-key blocks m=0..c
     (causal; the packed block m maps to global block 2m+h, so the count and
     the boundary structure are core-independent): score matmuls in groups of
     up to 4 blocks -> one ACT exp per group (scale folds the 64x64 weight
     scaling) -> boundary mask (a single constant [128,256] tile, only block
     m==c needs it) -> PV matmuls ACCUMULATE the whole chunk in one PSUM tile
     [65, 256] (no SBUF accumulator, no DVE adds).
  3. The chunk accumulator DMAs straight from PSUM to DRAM via the POOL DGE
     (last two chunks via HWDGE); host combines + normalizes + transposes.
"""

import numpy as np

import concourse.mybir as mybir
import concourse.tile as tile
from concourse import bacc
from concourse.bass_utils import run_bass_kernel_spmd

B, S, D, DK = 4, 4096, 1024, 64
NCORES = 8
CH = 256  # query rows per chunk
NCH = 16  # chunks per core (all rows)
KB = 2048  # own keys per core
JB = 128  # key block
NKB = KB // JB  # 16 own key blocks
DC = D // 128  # 8 contraction chunks
F32 = mybir.dt.float32
BF16 = mybir.dt.bfloat16
E3 = mybir.dt.float8e3
WSCALE = 64.0  # host pre-scales Wq/Wk by this; folded into exp scale
SCALE = 0.125 / (WSCALE * WSCALE)

_prog_cache = {}


def _build(variant):
    causal = variant == "causal"
    nkq = [c + 1 if causal else NKB for c in range(NCH)]  # own blocks/chunk

    nc = bacc.Bacc("TRN2", target_bir_lowering=False, debug=False,
                   num_devices=NCORES)

    qt_d = nc.declare_dram_parameter("qt", [D, S], E3, isOutput=False)
    kt_d = nc.declare_dram_parameter("kt", [D, KB], E3, isOutput=False)
    vt_d = nc.declare_dram_parameter("vt", [D, KB], BF16, isOutput=False)
    wq_d = nc.declare_dram_parameter("wq", [D, DK], E3, isOutput=False)
    wk_d = nc.declare_dram_parameter("wk", [D, DK], E3, isOutput=False)
    wv_d = nc.declare_dram_parameter("wv", [D, DK], BF16, isOutput=False)
    if causal:
        mask_d = nc.declare_dram_parameter("maskb", [JB, CH], BF16,
                                           isOutput=False)
    # raw transposed partial accumulators (+denominator row); host combines
    out_d = nc.declare_dram_parameter("out", [NCH, DK + 1, CH], F32,
                                      isOutput=True)

    NSQ = S // 512  # 8 column groups of 512 for q
    NSK = KB // 512  # 4 groups for k/v

    qt3 = qt_d.rearrange("(o p) s -> p o s", p=128)
    kt3 = kt_d.rearrange("(o p) s -> p o s", p=128)
    vt3 = vt_d.rearrange("(o p) s -> p o s", p=128)

    with tile.TileContext(nc) as tc:
        with (
            tc.tile_pool(name="const", bufs=1) as const,
            tc.tile_pool(name="res", bufs=1) as res,
            tc.tile_pool(name="stage", bufs=10) as stage,
            tc.tile_pool(name="pwork", bufs=4) as pwork,
            tc.tile_pool(name="ps_mm", bufs=2, space="PSUM") as ps_mm,
            tc.tile_pool(name="ps_s", bufs=2, space="PSUM") as ps_s,
            tc.tile_pool(name="ps_o", bufs=2, space="PSUM") as ps_o,
        ):
            def stage_load(src3, sc, dt, splits=2):
                """Split-group DMAs so the first matmuls start early."""
                w = DC // splits
                sts = []
                for hh in range(splits):
                    st = stage.tile([128, w, 512], dt, tag="stage",
                                    name=f"st{hh}")
                    nc.sync.dma_start(
                        st[:],
                        src3[:, w * hh:w * (hh + 1), sc * 512:(sc + 1) * 512])
                    sts.append(st)
                return sts

            bg = []  # background projection thunks, woven between attn units

            def project_qk(kind, src3, w_sb, dst, sc, sts=None, defer=False):
                """One 512-column group: 8 accumulating matmuls (weights
                stationary); psum copied to the bf16 qT/kT tile."""
                if sts is None:
                    sts = stage_load(src3, sc, E3)
                w = DC // len(sts)
                box = {}

                def mm(dc):
                    if dc == 0:
                        box["ps"] = ps_mm.tile([DK, 512], F32, tag="mm",
                                               name="ps_qk")
                    nc.tensor.matmul(box["ps"][:], w_sb[:, dc, :],
                                     sts[dc // w][:, dc % w, :],
                                     start=(dc == 0), stop=(dc == DC - 1))
                    if dc == DC - 1:
                        nc.vector.tensor_copy(dst[:], box["ps"][:])

                for dc in range(DC):
                    if defer:
                        bg.append(((kind, sc), lambda dc=dc: mm(dc)))
                    else:
                        mm(dc)

            def project_v(sc, sts=None, defer=False):
                """V projected directly to natural [s, c] blocks: lhsT is the
                staged data chunk, rhs the weights -> out [128 s, 64 c], which
                is exactly the PV lhsT layout."""
                if sts is None:
                    sts = stage_load(vt3, sc, BF16)
                w = DC // len(sts)
                box = {}

                def mm(sb):
                    if sb == 0:
                        box["ps"] = ps_mm.tile([128, 4, DK], F32, tag="mm",
                                               name="ps_v")
                    for dc in range(DC):
                        nc.tensor.matmul(
                            box["ps"][:, sb, :],
                            sts[dc // w][:, dc % w,
                                         sb * 128:(sb + 1) * 128],
                            wv_sb[:, dc, :],
                            start=(dc == 0), stop=(dc == DC - 1))
                    nc.vector.tensor_copy(vgs[sc][:, sb, 0:DK],
                                          box["ps"][:, sb, :])
                    if sb == 3:
                        nc.vector.memset(vgs[sc][:, :, DK:DK + 1], 1.0)

                for sb in range(4):
                    if defer:
                        bg.append((("v", sc), lambda sb=sb: mm(sb)))
                    else:
                        mm(sb)

            # PE warm-up in the initial DMA shadow
            warm = const.tile([128, 512], BF16, tag="warm")
            nc.vector.memset(warm[:], 0.0)
            for _ in range(8):
                wps = ps_mm.tile([DK, 512], F32, tag="mm", name="wps")
                nc.tensor.matmul(wps[:], warm[:, 0:DK], warm[:],
                                 start=True, stop=True)
            wq_sb = const.tile([128, DC, DK], E3, tag="wq")
            wk_sb = const.tile([128, DC, DK], E3, tag="wk")
            wv_sb = const.tile([128, DC, DK], BF16, tag="wv")
            head_q0 = stage_load(qt3, 0, E3)
            nc.sync.dma_start(wq_sb[:], wq_d.rearrange("(o p) c -> p o c", p=128))
            nc.sync.dma_start(wk_sb[:], wk_d.rearrange("(o p) c -> p o c", p=128))
            nc.sync.dma_start(wv_sb[:], wv_d.rearrange("(o p) c -> p o c", p=128))
            head_k0 = stage_load(kt3, 0, E3)
            head_v0 = stage_load(vt3, 0, BF16)
            if causal:
                msk_sb = const.tile([JB, CH], BF16, tag="msk")
                nc.sync.dma_start(msk_sb[:], mask_d[:])

            # qT tiles [64, 512] bf16 (2 chunks per tile)
            qts = [res.tile([DK, 512], BF16, tag=f"qt{sc}", name=f"qt{sc}")
                   for sc in range(NSQ)]
            # kT tiles [64, 512] (4 own key blocks per tile)
            kts = [res.tile([DK, 512], BF16, tag=f"kt{sc}", name=f"kt{sc}")
                   for sc in range(NSK)]
            # v natural (+ones col): per 512-group, 4 blocks of [128, 65]
            vgs = [res.tile([128, 4, DK + 1], BF16, tag=f"vg{sc}",
                            name=f"vg{sc}")
                   for sc in range(NSK)]

            def q_rhs(c):
                return qts[c // 2][:, (c % 2) * CH:(c % 2 + 1) * CH]

            def emit_unit(c, m0, nb, o_ps, first):
                nb_tot = nkq[c]
                s_ps = ps_s.tile([128, nb, CH], F32, tag="s", name=f"s{nb}")
                for i in range(nb):
                    m = m0 + i
                    nc.tensor.matmul(
                        s_ps[:, i, :],
                        kts[m // 4][:, (m % 4) * JB:(m % 4 + 1) * JB],
                        q_rhs(c), start=True, stop=True)
                if bg:
                    bg.pop(0)[1]()
                p_sb = pwork.tile([128, nb, CH], BF16, tag="p",
                                  name=f"p{nb}")
                nc.scalar.activation(p_sb[:], s_ps[:],
                                     mybir.ActivationFunctionType.Exp,
                                     scale=SCALE)
                if causal and m0 + nb == nb_tot:
                    # boundary block is always the chunk's last block
                    nc.vector.tensor_mul(p_sb[:, nb - 1, :],
                                         p_sb[:, nb - 1, :], msk_sb[:])
                for i in range(nb):
                    m = m0 + i
                    nc.tensor.matmul(
                        o_ps[:], vgs[m // 4][:, m % 4, :], p_sb[:, i, :],
                        start=(first and i == 0), stop=(m == nb_tot - 1))

            def epilogue(c, o_ps):
                # POOL DGE so result stores don't head-of-line block the SP
                # sequencer issuing input stage loads
                o_sb = pwork.tile([DK + 1, CH], F32, tag="osb", name="o_sb")
                nc.vector.tensor_copy(o_sb[:], o_ps[:])
                eng = nc.sync if c >= NCH - 2 else nc.gpsimd
                eng.dma_start(out_d[c], o_sb[:])

            def chunk_pair(c0, c1):
                """Interleave the score/exp/PV units of two chunks so one
                chunk's PE work hides the other's ACT-exp latency (each chunk
                accumulates in its own PSUM bank)."""
                cs = [c for c in (c0, c1) if c is not None]
                units = {c: [(m0, min(4, nkq[c] - m0))
                             for m0 in range(0, nkq[c], 4)] for c in cs}
                ops = {c: ps_o.tile([DK + 1, CH], F32, tag="o",
                                    name=f"o{c % 2}") for c in cs}
                nu = max(len(units[c]) for c in cs)
                for u in range(nu):
                    for c in cs:
                        if u < len(units[c]):
                            m0, nb = units[c][u]
                            emit_unit(c, m0, nb, ops[c], first=(u == 0))
                for c in cs:
                    epilogue(c, ops[c])

            # projection prefetch schedule: group g of k/v feeds chunks
            # >= 4g, q group g feeds chunks >= 2g; emit ~2 chunks early
            pre = {c: [] for c in range(NCH)}
            for g in range(1, NSK):  # k/v groups 1..3 needed at chunk 4g
                pre[max(0, 4 * g - 3)] += [("k", g), ("v", g)]
            for g in range(1, NSQ):  # q groups 1..7 needed at chunk 2g
                pre[max(0, 2 * g - 2)] += [("q", g)]

            project_qk("q", qt3, wq_sb, qts[0], 0, sts=head_q0)
            project_qk("k", kt3, wk_sb, kts[0], 0, sts=head_k0)
            project_v(0, sts=head_v0)
            for c0 in range(0, NCH, 2):
                c1 = c0 + 1
                # groups the current pair depends on must be fully emitted
                needed = {("q", g) for g in range(c1 // 2 + 1)}
                needed |= {(kd, g) for g in range(c1 // 4 + 1)
                           for kd in ("k", "v")}
                while bg and bg[0][0] in needed:
                    bg.pop(0)[1]()
                # stage + enqueue projections for upcoming chunks; their
                # matmuls are woven between this pair's attention units
                for c in (c0, c1):
                    for kind, g in pre[c]:
                        if kind == "q":
                            project_qk("q", qt3, wq_sb, qts[g], g, defer=True)
                        elif kind == "k":
                            project_qk("k", kt3, wk_sb, kts[g], g, defer=True)
                        else:
                            project_v(g, defer=True)
                chunk_pair(c0, c1)
            while bg:
                bg.pop(0)[1]()

    nc.compile()
    return nc


def _get_prog(variant):
    if variant not in _prog_cache:
        _prog_cache[variant] = _build(variant)
    return _prog_cache[variant]


def kernel(queries, keys, values, Wq, Wk, Wv, mask):
    import ml_dtypes  # noqa: F401  registers numpy bfloat16/fp8

    bf16 = np.dtype(mybir.dt.np(BF16))
    e3m4 = np.dtype(mybir.dt.np(E3))
    queries = np.asarray(queries, dtype=np.float32)
    keys = np.asarray(keys, dtype=np.float32)
    values = np.asarray(values, dtype=np.float32)
    mask_np = np.asarray(mask)

    causal = bool(np.array_equal(
        mask_np != 0, np.tril(np.ones((S, S), dtype=bool))))
    full = bool((mask_np != 0).all()) if not causal else False
    if not (causal or full):
        raise NotImplementedError("general mask not supported")
    variant = "causal" if causal else "full"

    qt = np.ascontiguousarray(queries.transpose(0, 2, 1)).astype(e3m4)
    kt = np.ascontiguousarray(keys.transpose(0, 2, 1)).astype(e3m4)
    vt = np.ascontiguousarray(values.transpose(0, 2, 1)).astype(bf16)
    wq = np.ascontiguousarray(
        np.asarray(Wq, dtype=np.float32).T * WSCALE).astype(e3m4)
    wk = np.ascontiguousarray(
        np.asarray(Wk, dtype=np.float32).T * WSCALE).astype(e3m4)
    wv = np.ascontiguousarray(np.asarray(Wv, dtype=np.float32).T).astype(bf16)

    in_maps = []
    for core in range(NCORES):
        b, h = divmod(core, 2)
        ksel = np.ascontiguousarray(
            kt[b].reshape(D, S // JB, JB)[:, h::2, :].reshape(D, KB))
        vsel = np.ascontiguousarray(
            vt[b].reshape(D, S // JB, JB)[:, h::2, :].reshape(D, KB))
        m = {"qt": qt[b], "kt": ksel, "vt": vsel,
             "wq": wq, "wk": wk, "wv": wv}
        if variant == "causal":
            i = np.arange(CH)[None, :]
            j = np.arange(JB)[:, None]
            m["maskb"] = ((i - j - JB * h) >= 0).astype(np.float32).astype(bf16)
        in_maps.append(m)

    nc = _get_prog(variant)
    res = run_bass_kernel_spmd(nc, in_maps, list(range(NCORES)))

    out = np.empty((B, S, DK), dtype=np.float32)
    ov = out.reshape(B, NCH, CH, DK)
    for b in range(B):
        r0 = res.results[2 * b]["out"]  # [NCH, DK+1, CH]
        r1 = res.results[2 * b + 1]["out"]
        tot = r0.astype(np.float64) + r1.astype(np.float64)
        ov[b] = (tot[:, :DK, :] / tot[:, DK:DK + 1, :]).transpose(0, 2, 1)
    return out


if __name__ == "__main__":
    rng = np.random.default_rng(0)
    q = rng.standard_normal((B, S, D), dtype=np.float32)
    k = rng.standard_normal((B, S, D), dtype=np.float32)
    v = rng.standard_normal((B, S, D), dtype=np.float32)
    sc = 1.0 / np.sqrt(D)
    wq = rng.uniform(-sc, sc, (DK, D)).astype(np.float32)
    wk = rng.uniform(-sc, sc, (DK, D)).astype(np.float32)
    wv = rng.uniform(-sc, sc, (DK, D)).astype(np.float32)
    msk = np.tril(np.ones((S, S), dtype=np.int32))
    out = kernel(queries=q, keys=k, values=v, Wq=wq, Wk=wk, Wv=wv, mask=msk)
    print("out", out.shape, out.dtype, float(np.abs(out).mean()))
